# revision 30
# baseline (speedup 1.0000x reference)
"""Single-head causal self-attention (B=8, T=2048, D=512, H=64), data-parallel
over batch across 8 NeuronCores. Self-contained: builds a Bass/Tile kernel and
runs it via run_bass_kernel_spmd.

v4 design (per core, batch element b = core id), all-bf16 compute:
  - host prep: x transposed + cast to bf16 (xT [4,128,2048]); Wk|Wq*s|Wv
    packed [4,128,192] bf16; biases pre-broadcast (bkqB [128,128],
    bvB [128,64]); triangular mask tile [128,2,128]
  - projections per t-tile: stationary xT chunk [128d,128t] x moving W
    [128d,192] -> psum [128t, k|q|v]; biases folded into the psum->sbuf
    copies (k|q staged for transpose, v+bv -> v_aug); k,q PE-transposed
    (bf16) into [64,2tile] psum strips, then one plain copy to kqT sbuf
  - attention per 256-wide i-block ib (j-tiles 0..2ib+1): S^T groups in
    flat [128,<=1024] psum (no slot crosses a 512-col bank boundary):
    groups of 4 normal tiles; final group = <=2 normals + diag-even full +
    diag-odd right-half (left half fully masked -> skipped); one exp per
    group (ACT) -> e2 bf16; triangular mask multiply on the two diagonal
    128-col pieces (DVE, one strided op)
  - PV: e2 128-col chunks stationary x v_aug [128,65] (v+bv | 1) moving ->
    [t,65] psum; single psum bank [128,2,65]: i-tile c=0 accumulates first
    (group-lagged), then c=1 as a burst - the two accumulation groups are
    temporally disjoint so they share the bank
  - epilogue per i-block: batched reciprocal of ones-columns, scale, DMA out
  - emission is software-pipelined (P/S interleaved, V lagged behind exp);
    the tile scheduler further list-schedules per engine
"""

import sys
from collections import deque

for _p in ("/root/.axon_site/_ro/trn_rl_repo", "/opt/trn_rl_repo"):
    if _p not in sys.path:
        sys.path.append(_p)

import numpy as np
import concourse.bass as bass
import concourse.bacc as bacc
import concourse.tile as tile
from concourse import mybir
from concourse.bass_utils import run_bass_kernel_spmd
from concourse.masks import make_identity

F32 = mybir.dt.float32
BF16 = mybir.dt.bfloat16

B, T, D, H = 8, 2048, 512, 64
NT = T // 128   # 16 t-tiles
ND = D // 128   # 4 d-chunks
NIB = T // 256  # 8 i-blocks
EXP = mybir.ActivationFunctionType.Exp
MULT = mybir.AluOpType.mult
ADD = mybir.AluOpType.add


def attn_groups(ib):
    """Slot lists per group: (jt, col0, width, cs). Normal tiles fill
    groups of 4; final group = <=2 normals + diag-even (256) + diag-odd
    right half (128)."""
    nrm = list(range(2 * ib))
    groups = []
    while len(nrm) > 2:
        groups.append([(nrm.pop(0), i * 256, 256, (0, 1)) for i in range(4)])
    last, col = [], 0
    while nrm:
        last.append((nrm.pop(0), col, 256, (0, 1)))
        col += 256
    last.append((2 * ib, col, 256, (0, 1)))          # diag-even
    last.append((2 * ib + 1, col + 256, 128, (1,)))  # diag-odd right half
    groups.append(last)
    return groups


def build_body(nc, tc, ctx, dram, repeat=1):
    xT_d, const_d, out_d = dram

    persist = ctx.enter_context(tc.tile_pool(name="persist", bufs=1))
    stg = ctx.enter_context(tc.tile_pool(name="stg", bufs=3))
    e2pool = ctx.enter_context(tc.tile_pool(name="e2", bufs=6))
    recpool = ctx.enter_context(tc.tile_pool(name="rec", bufs=2))
    # persistent psum: proj kqv (2 regions x 192 in one bank) and kq-transpose
    # strips (2 strips x [64,2,256]bf16 in one bank), manually double-buffered
    psP = ctx.enter_context(tc.tile_pool(name="psP", bufs=1, space="PSUM"))
    psS = ctx.enter_context(tc.tile_pool(name="psS", bufs=2, space="PSUM"))
    psO = ctx.enter_context(tc.tile_pool(name="psO", bufs=2, space="PSUM"))
    ps_kqv = psP.tile([128, 2, 3 * H], F32, tag="kqv")
    strips = psP.tile([64, 2, 2, 256], BF16, tag="kqT")

    identB = persist.tile([128, 128], BF16)

    xT = persist.tile([128, ND, T], BF16)
    # consts packed in one [128, 1216] bf16 block:
    # w [4*192=768] | bkqB [128] | bvB [64] | masks [256]
    consts = persist.tile([128, 1216], BF16)
    w_sb = consts[:, 0:768].rearrange("p (a h) -> p a h", a=ND)
    bkqB = consts[:, 768:896]
    bvB = consts[:, 896:960]
    masks = consts[:, 960:1216].rearrange("p (a c) -> p a c", a=2)
    kqT = persist.tile([64, 2, T], BF16)       # [h, {k,q}, t]
    v_aug = persist.tile([128, NT, 65], BF16)  # v rows (+bv) | ones column
    o_all = persist.tile([128, NT, 64], F32)

    # HWDGE descriptor generation is serialized (~630ns per dma_start), so:
    # x tiles 0-1 take the first HWDGE slot (sync); the consts block goes
    # via gpsimd SWDGE which runs in parallel; big x spans follow.
    def dma_x(eng, t0, ntile):
        eng.dma_start(
            xT[:, :, 128 * t0:128 * (t0 + ntile)],
            xT_d[:, :, 128 * t0:128 * (t0 + ntile)].rearrange("a p t -> p a t"))

    dma_x(nc.sync, 0, 2)
    nc.gpsimd.dma_start(consts[:], const_d[:])
    dma_x(nc.scalar, 2, 6)
    dma_x(nc.sync, 8, 8)
    make_identity(nc, identB[:])

    def unit_P(tt):
        ps = ps_kqv[:, tt % 2, :]
        for dc in range(ND):
            nc.tensor.matmul(ps, xT[:, dc, tt * 128:(tt + 1) * 128],
                             w_sb[:, dc, :], start=(dc == 0), stop=(dc == ND - 1))
        st = stg.tile([128, 128], BF16, tag="stage", name=f"st{tt}")
        nc.vector.tensor_tensor(st[:], ps[:, 0:128], bkqB[:], ADD)
        nc.vector.tensor_tensor(v_aug[:, tt, 0:64], ps[:, 128:192], bvB[:], ADD)
        strip = strips[:, (tt // 2) % 2, :, :]
        half = (tt % 2) * 128
        nc.tensor.transpose(strip[:, 0, half:half + 128], st[:, 0:64], identB[:])
        nc.tensor.transpose(strip[:, 1, half:half + 128], st[:, 64:128], identB[:])
        if tt % 2 == 1:
            nc.vector.tensor_copy(kqT[:, :, (tt - 1) * 128:(tt + 1) * 128],
                                  strip)

    e2_of = {}     # (ib, gi) -> e2 tile

    def unit_S(ib, gi):
        groups = attn_groups(ib)
        group = groups[gi]
        ncols = group[-1][1] + group[-1][2]
        ps = psS.tile([128, 1024], F32, tag="S", name=f"s{ib}_{gi}")
        for jt, col0, w, _cs in group:
            ioff = ib * 256 + (128 if w == 128 else 0)
            nc.tensor.matmul(ps[:, col0:col0 + w],
                             kqT[:, 0, jt * 128:(jt + 1) * 128],
                             kqT[:, 1, ioff:ioff + w], start=True, stop=True)
        e2 = e2pool.tile([128, 1024], BF16, tag="e2", name=f"e{ib}_{gi}")
        nc.scalar.activation(e2[:, 0:ncols], ps[:, 0:ncols], EXP)
        if gi == len(groups) - 1:
            moff = group[-2][1]
            sel = e2[:, moff:moff + 384].rearrange(
                "p (a c) -> p a c", c=128)[:, 0::2, :]
            nc.vector.tensor_tensor(sel, sel, masks[:], MULT)
        e2_of[(ib, gi)] = e2

    po_of = {}

    def pv_slots(ib, c):
        """(gi, jt, coff) for every PV matmul of column c, in group order."""
        out = []
        for gi, grp in enumerate(attn_groups(ib)):
            for jt, col0, w, cs in grp:
                if c in cs:
                    out.append((gi, jt, col0 + (0 if w == 128 else c * 128)))
        return out

    def unit_V0(ib, gi):
        """PV for group gi, both i-tile columns (po pair in separate banks)."""
        if gi == 0:
            po_of[ib] = [psO.tile([128, 65], F32, tag="O", name=f"po{ib}_{c}")
                         for c in range(2)]
        po = po_of[ib]
        for c in (0, 1):
            slots = pv_slots(ib, c)
            for g, jt, coff in [s for s in slots if s[0] == gi]:
                nc.tensor.matmul(po[c][:], e2_of[(ib, gi)][:, coff:coff + 128],
                                 v_aug[:, jt, :],
                                 start=((gi, jt, coff) == slots[0]),
                                 stop=((gi, jt, coff) == slots[-1]))

    def unit_V1E(ib):
        """Epilogue + output DMA."""
        po = po_of.pop(ib)
        for gi in range(len(attn_groups(ib))):
            e2_of.pop((ib, gi))
        for c in range(2):
            rec = recpool.tile([128, 1], F32, tag="rec", name=f"rec{ib}_{c}")
            nc.vector.reciprocal(rec[:], po[c][:, 64:65])
            nc.vector.tensor_scalar_mul(o_all[:, 2 * ib + c, :],
                                        po[c][:, 0:64], rec[:])
        nc.sync.dma_start(
            out_d[256 * ib:256 * (ib + 1), :].rearrange("(a p) h -> p a h", p=128),
            o_all[:, 2 * ib:2 * ib + 2, :])

    def emit_schedule():
        s_avail = deque()
        v_pend = deque()
        s_emitted = 0
        v0_done = [0] * NIB

        def drain_v(lag):
            while v_pend and v_pend[0][0] <= s_emitted - lag:
                _, ib, gi = v_pend.popleft()
                unit_V0(ib, gi)
                v0_done[ib] += 1
                if v0_done[ib] == len(attn_groups(ib)):
                    unit_V1E(ib)

        for tt in range(NT):
            unit_P(tt)
            if tt % 2 == 1:
                ib = tt // 2
                for gi in range(len(attn_groups(ib))):
                    s_avail.append((ib, gi))
            if s_avail:
                ib, gi = s_avail.popleft()
                unit_S(ib, gi)
                s_emitted += 1
                v_pend.append((s_emitted, ib, gi))
            drain_v(2)
        while s_avail:
            ib, gi = s_avail.popleft()
            unit_S(ib, gi)
            s_emitted += 1
            v_pend.append((s_emitted, ib, gi))
            drain_v(2)
        drain_v(0)

    for rep in range(repeat):
        nc.vector.memset(v_aug[:, :, 64:65], 1.0)
        emit_schedule()


def build_nc(repeat=1):
    nc = bacc.Bacc("TRN2", target_bir_lowering=False, debug=False, num_devices=8)
    xT_d = nc.dram_tensor("xT", [ND, 128, T], BF16, kind="ExternalInput")
    const_d = nc.dram_tensor("const", [128, 1216], BF16, kind="ExternalInput")
    out_d = nc.dram_tensor("out", [T, H], F32, kind="ExternalOutput")
    dram = (xT_d, const_d, out_d)

    from contextlib import ExitStack
    with tile.TileContext(nc) as tc:
        with ExitStack() as ctx:
            build_body(nc, tc, ctx, dram, repeat=repeat)
    nc.compile()
    return nc


_NC_CACHE = {}


def _get_nc(repeat=1):
    if repeat not in _NC_CACHE:
        _NC_CACHE[repeat] = build_nc(repeat)
    return _NC_CACHE[repeat]


def make_in_maps(x, Wk, bk, Wq, bq, Wv, bv):
    import ml_dtypes
    bf16 = ml_dtypes.bfloat16
    scale = float(H) ** -0.5
    w = np.concatenate([Wk, Wq * scale, Wv], axis=1)          # [512, 192]
    # packed consts [128, 1216]: w-chunks | bkqB | bvB | masks
    w128 = w.reshape(ND, 128, 3 * H).transpose(1, 0, 2).reshape(128, 768)
    bkq = np.concatenate([bk, bq * scale])                    # [128]
    bkqB = np.broadcast_to(bkq, (128, 128))
    bvB = np.broadcast_to(bv, (128, H))
    r = np.arange(128)[:, None]
    c = np.arange(128)[None, :]
    m = (c >= r)
    masks = np.stack([m, m], axis=1).reshape(128, 256)        # [128, 2*128]
    consts = np.ascontiguousarray(
        np.concatenate([w128, bkqB, bvB, masks], axis=1)).astype(bf16)
    ins = []
    for b in range(B):
        xTb = np.ascontiguousarray(x[b].T).astype(bf16).reshape(ND, 128, T)
        ins.append({"xT": xTb, "const": consts})
    return ins


def kernel(x, Wk, bk, Wq, bq, Wv, bv, _repeat=1):
    x = np.asarray(x, dtype=np.float32)
    Wk = np.asarray(Wk, dtype=np.float32)
    bk = np.asarray(bk, dtype=np.float32)
    Wq = np.asarray(Wq, dtype=np.float32)
    bq = np.asarray(bq, dtype=np.float32)
    Wv = np.asarray(Wv, dtype=np.float32)
    bv = np.asarray(bv, dtype=np.float32)

    nc = _get_nc(_repeat)
    in_maps = make_in_maps(x, Wk, bk, Wq, bq, Wv, bv)
    res = run_bass_kernel_spmd(nc, in_maps, core_ids=list(range(B)))
    out = np.stack([res.results[b]["out"] for b in range(B)], axis=0)
    return out.astype(np.float32)
